# revision 10
# baseline (speedup 1.0000x reference)
"""Trainium2 Bass kernel for nn_Brick_Wall (brick-wall gate-layer gradient).

Math: for each gate g the 4x4 antisymmetric E(chi) splits over so(4) =
su(2)+su(2) as E = L(a) + R(b) (left/right quaternion multiplications), so
expm(E) = L(exp_H a) R(exp_H b) in closed form (sin/cos), and the Frechet
derivative d expm(E)[D_m] contracts against the per-gate matrix
Z = (W C^T - C^T W) U down to two per-gate 4-vectors kappa/lambda:
    partials[m] = dp_m . kappa + dq_m . lambda
with dp_m/dq_m given by the derivative of the quaternion exp in closed form.

Sharding: gates (2048) split contiguously across 8 cores (256 = 2 blocks of
128 partitions each). Host does layout marshaling only (diag-block extraction,
signed column permutations, reshapes); all arithmetic runs on-device.
"""
import sys

for _p in ("/opt/trn_rl_repo",):
    if _p not in sys.path:
        sys.path.insert(0, _p)

import numpy as np

import concourse.bacc as bacc
import concourse.bass as bass
import concourse.tile as tile
from concourse import mybir
from concourse.bass_utils import run_bass_kernel_spmd

F32 = np.float32
P = 128          # partitions (gates per block)
B = 2            # gate blocks per core
NCORES = 8
GPC = P * B      # gates per core
PI = float(np.pi)
DT = mybir.dt.float32

# ---------------- constant tables (quaternion algebra) ----------------
_Q = np.zeros((4, 4, 4))
for (a, b), (c, s) in {
    (0, 0): (0, 1), (0, 1): (1, 1), (0, 2): (2, 1), (0, 3): (3, 1),
    (1, 0): (1, 1), (1, 1): (0, -1), (1, 2): (3, 1), (1, 3): (2, -1),
    (2, 0): (2, 1), (2, 1): (3, -1), (2, 2): (0, -1), (2, 3): (1, 1),
    (3, 0): (3, 1), (3, 1): (2, 1), (3, 2): (1, -1), (3, 3): (0, -1),
}.items():
    _Q[a, b, c] = s

G_SGN = np.zeros((4, 4))   # R(qbar)[k,j] = G_SGN[k,j] * q_{k xor j}
H_SGN = np.zeros((4, 4))   # L(pbar)[i,k] = H_SGN[k,i] * p_{i xor k}
SL = np.zeros((4, 4))      # kappa_a = sum_j SL[a^j, j] * G[a^j, j]
SR = np.zeros((4, 4))      # lambda_b = sum_j SR[b^j, j] * H[b^j, j]
for k in range(4):
    for j in range(4):
        a = k ^ j
        G_SGN[k, j] = _Q[j, a, k] * (1 if a == 0 else -1)
        H_SGN[k, j] = _Q[a, k, j] * (1 if a == 0 else -1)
for a in range(4):
    for j in range(4):
        SL[a ^ j, j] = _Q[a, j, a ^ j]
for b in range(4):
    for j in range(4):
        SR[b ^ j, j] = _Q[j, b, b ^ j]

# internal direction order m' -> chi index; c(m')-1 = (0,0,1,1,2,2)
MPRIME = [4, 5, 1, 2, 0, 3]
SA = [1.0, 1.0, -1.0, -1.0, 1.0, -1.0]
SB = [1.0, -1.0, 1.0, -1.0, -1.0, -1.0]

# XOR gather: row k of the idx table (k^0, k^1, k^2, k^3) as offset + 2D AP
XOR_AP = {0: (0, 2, 1), 1: (1, 2, -1), 2: (2, -2, 1), 3: (3, -2, -1)}
# kappa/lambda reduce position sets {4*(a^j)+j} as offset + 2D strides
KPOS_AP = {0: (0, 10, 5), 1: (1, 10, 3), 2: (2, 6, 5), 3: (3, 6, 3)}

NCONST = 80


def _const_row() -> np.ndarray:
    c = np.zeros((1, NCONST), F32)
    c[0, 0:16] = SL.reshape(16)
    c[0, 16:32] = SR.reshape(16)
    c[0, 32:38] = SA
    c[0, 38:44] = SB
    c[0, 44:60] = G_SGN.reshape(16)
    c[0, 60:76] = H_SGN.reshape(16)
    c[0, 76] = -PI
    c[0, 77] = PI / 2
    return c


def _ap(base: bass.AP, off: int, *dims) -> bass.AP:
    """Rebuild an AP over `base`'s tensor: partition dim kept, free dims given
    as (stride, size) pairs, offset in elements added to base offset."""
    return bass.AP(tensor=base.tensor, offset=base.offset + off,
                   ap=[base.ap[0]] + [[s, n] for (s, n) in dims])


def tile_body(ctx, tc, outs, ins):
    """ins: ab(128,12), cb(128,32), ub(128,32), pp(128,4), cst(1,80)
    outs: res(128,12)  [B,6] per partition, internal m' order, sign-applied."""
    nc = tc.nc
    A = mybir.AluOpType
    AF = mybir.ActivationFunctionType
    ab_d, cb_d, ub_d, pp_d, cst_d = ins
    res_d = outs[0]

    pool = ctx.enter_context(tc.tile_pool(name="main", bufs=1))

    def T(tag, *shape):
        return pool.tile([P, *shape], DT, tag=tag, name=tag)

    # ---- DMA in ----
    ab = T("ab", B, 2, 3)
    cb = T("cb", B, 16)
    ub = T("ub", B, 16)
    pp = T("pp", B, 2)
    cst = T("cst", NCONST)
    nc.sync.dma_start(ab[:].rearrange("p a b c -> p (a b c)"), ab_d)
    nc.sync.dma_start(cb[:].rearrange("p a b -> p (a b)"), cb_d)
    nc.sync.dma_start(ub[:].rearrange("p a b -> p (a b)"), ub_d)
    nc.sync.dma_start(pp[:].rearrange("p a b -> p (a b)"), pp_d)
    nc.sync.dma_start(cst[:], bass.AP(tensor=cst_d.tensor, offset=cst_d.offset,
                                      ap=[[0, P], [1, NCONST]]))
    mpi = cst[:, 76:77]                       # -pi bias AP [P,1]
    hpi = cst[:, 77:78]                       # +pi/2 bias AP [P,1]

    # ---- S1: w = [a; b] = [al+be; al-be]   w[B,2,3] ----
    w = T("w", B, 2, 3)
    nc.vector.tensor_add(w[:, :, 0, :], ab[:, :, 0, :], ab[:, :, 1, :])
    nc.vector.tensor_sub(w[:, :, 1, :], ab[:, :, 0, :], ab[:, :, 1, :])

    # ---- S2: per-gate scalars  (tiles [B,2]) ----
    wsq = T("wsq", B, 2, 3)
    nc.vector.tensor_mul(wsq[:], w[:], w[:])
    h2 = T("h2", B, 2)
    nc.vector.tensor_reduce(out=_ap(h2[:], 0, (2, B), (1, 2), (0, 1)),
                            in_=wsq[:], axis=mybir.AxisListType.X, op=A.add)
    h = T("h", B, 2)
    nc.scalar.sqrt(h[:], h2[:])
    ih2 = T("ih2", B, 2)
    nc.vector.reciprocal(ih2[:], h2[:])
    ih = T("ih", B, 2)
    nc.vector.tensor_mul(ih[:], h[:], ih2[:])
    # range reduction: r = h - 2pi*round(h/2pi) in [-pi, pi], via magic-number
    # rounding (adding 1.5*2^23 forces RNE to integer for |y| < 2^22)
    MAGIC = 12582912.0
    ym = T("ym", B, 2)
    nc.vector.tensor_scalar(ym[:], h[:], 1.0 / (2 * PI), MAGIC, op0=A.mult, op1=A.add)
    rnd = T("rnd", B, 2)
    nc.vector.tensor_scalar(rnd[:], ym[:], -MAGIC, None, op0=A.add)
    rr = T("rr", B, 2)
    nc.vector.scalar_tensor_tensor(rr[:], rnd[:], -2 * PI, h[:], op0=A.mult, op1=A.add)
    sin = T("sin", B, 2)
    nc.scalar.activation(sin[:], rr[:], AF.Sin)
    ra = T("ra", B, 2)
    nc.scalar.activation(ra[:], rr[:], AF.Abs)
    cos = T("cos", B, 2)
    nc.scalar.activation(cos[:], ra[:], AF.Sin, bias=hpi, scale=-1.0)
    snc = T("snc", B, 2)
    nc.vector.tensor_mul(snc[:], sin[:], ih[:])
    s1t = T("s1t", B, 2)
    nc.vector.scalar_tensor_tensor(s1t[:], sin[:], -1.0, ih[:], op0=A.mult, op1=A.mult)
    dcs = T("dcs", B, 2)
    nc.vector.tensor_sub(dcs[:], cos[:], snc[:])
    s2t = T("s2t", B, 2)
    nc.vector.tensor_mul(s2t[:], dcs[:], ih2[:])

    # ---- S3: quaternions pq[B,2,4] (pm0=p from a, pm1=q from b) ----
    pq = T("pq", B, 2, 4)
    nc.scalar.copy(_ap(pq[:], 0, (8, B), (4, 2), (0, 1)), _ap(cos[:], 0, (2, B), (1, 2), (0, 1)))
    nc.vector.tensor_tensor(_ap(pq[:], 1, (8, B), (4, 2), (1, 3)),
                            _ap(snc[:], 0, (2, B), (1, 2), (0, 3)),
                            w[:], op=A.mult)

    # ---- S4: Z = (W C^T - C^T W) U  via rank-1 structure ----
    # vprod[c,j,k] = C[k, 2c] * U[k, j]; v[c,j] = sum_k
    vprod = T("vprod", B, 2, 4, 4)
    for c in range(2):
        nc.vector.tensor_tensor(vprod[:, :, c],
                                _ap(cb[:], 2 * c, (16, B), (0, 4), (4, 4)),
                                _ap(ub[:], 0, (16, B), (1, 4), (4, 4)),
                                op=A.mult)
    v = T("v", B, 2, 4)
    nc.vector.tensor_reduce(out=_ap(v[:], 0, (8, B), (4, 2), (1, 4), (0, 1)),
                            in_=vprod[:], axis=mybir.AxisListType.X, op=A.add)
    Zr = T("Zr", B, 16)
    nc.gpsimd.memset(Zr[:], 0.0)
    nc.vector.tensor_tensor(_ap(Zr[:], 4, (16, B), (8, 2), (1, 4)),
                            _ap(pp[:], 0, (2, B), (1, 2), (0, 4)),
                            v[:], op=A.mult)
    sc = T("sc", B, 2, 4)     # sc[t,i] = -pp_t * C[2t+1, i]
    nc.vector.scalar_tensor_tensor(sc[:],
                                   _ap(pp[:], 0, (2, B), (1, 2), (0, 4)), -1.0,
                                   _ap(cb[:], 4, (16, B), (8, 2), (1, 4)),
                                   op0=A.mult, op1=A.mult)
    P2 = T("P2", B, 4, 4, 2)  # (i,j,t) = sc[t,i]*U[2t,j]
    for t in range(2):
        nc.vector.tensor_tensor(_ap(P2[:], t, (32, B), (8, 4), (2, 4)),
                                _ap(sc[:], 4 * t, (8, B), (1, 4), (0, 4)),
                                _ap(ub[:], 8 * t, (16, B), (0, 4), (1, 4)),
                                op=A.mult)
    Zc = T("Zc", B, 16)
    nc.vector.tensor_reduce(out=_ap(Zc[:], 0, (16, B), (4, 4), (1, 4), (0, 1)),
                            in_=P2[:], axis=mybir.AxisListType.X, op=A.add)
    Z = T("Z", B, 16)
    nc.vector.tensor_add(Z[:], Zr[:], Zc[:])

    # ---- signed quaternion copies ----
    qpm = T("qpm", B, 8)
    nc.scalar.copy(_ap(qpm[:], 0, (8, B), (1, 4)), pq[:, :, 1, :])
    nc.scalar.mul(_ap(qpm[:], 4, (8, B), (1, 4)), pq[:, :, 1, :], -1.0)
    ppm = T("ppm", B, 8)
    nc.scalar.copy(_ap(ppm[:], 0, (8, B), (1, 4)), pq[:, :, 0, :])
    nc.scalar.mul(_ap(ppm[:], 4, (8, B), (1, 4)), pq[:, :, 0, :], -1.0)

    # ---- G = Z @ R(qbar) ----
    Rq = T("Rq", B, 16)
    for k in range(4):
        off, sA_, sB_ = XOR_AP[k]
        nc.vector.tensor_tensor(_ap(Rq[:], 4 * k, (16, B), (1, 4)),
                                _ap(qpm[:], off, (8, B), (sA_, 2), (sB_, 2)),
                                _ap(cst[:], 44 + 4 * k, (0, B), (1, 4)),
                                op=A.mult)
    # term_k = Zcol_k (x) Rq_row_k ; tree-accumulate with 3 adds
    Gm = T("Gm", B, 16)
    t0, t1 = T("gtm0", B, 16), T("gtm1", B, 16)
    ta, tb = T("gta", B, 16), T("gtb", B, 16)
    nc.vector.tensor_tensor(t0[:], _ap(Z[:], 0, (16, B), (4, 4), (0, 4)),
                            _ap(Rq[:], 0, (16, B), (0, 4), (1, 4)), op=A.mult)
    nc.vector.tensor_tensor(t1[:], _ap(Z[:], 1, (16, B), (4, 4), (0, 4)),
                            _ap(Rq[:], 4, (16, B), (0, 4), (1, 4)), op=A.mult)
    nc.vector.tensor_add(ta[:], t0[:], t1[:])
    nc.vector.tensor_tensor(t0[:], _ap(Z[:], 2, (16, B), (4, 4), (0, 4)),
                            _ap(Rq[:], 8, (16, B), (0, 4), (1, 4)), op=A.mult)
    nc.vector.tensor_tensor(t1[:], _ap(Z[:], 3, (16, B), (4, 4), (0, 4)),
                            _ap(Rq[:], 12, (16, B), (0, 4), (1, 4)), op=A.mult)
    nc.vector.tensor_add(tb[:], t0[:], t1[:])
    nc.vector.tensor_add(Gm[:], ta[:], tb[:])

    # ---- H = L(pbar) @ Z  (on gpsimd to run in parallel with G on DVE) ----
    Lp = T("Lp", B, 16)
    for k in range(4):
        off, sA_, sB_ = XOR_AP[k]
        nc.gpsimd.tensor_tensor(_ap(Lp[:], k, (16, B), (4, 4)),
                                _ap(ppm[:], off, (8, B), (sA_, 2), (sB_, 2)),
                                _ap(cst[:], 60 + 4 * k, (0, B), (1, 4)),
                                op=A.mult)
    u0, u1 = T("htm0", B, 16), T("htm1", B, 16)
    ua, ub_t = T("hta", B, 16), T("htb", B, 16)
    Hm = T("Hm", B, 16)
    nc.gpsimd.tensor_tensor(u0[:], _ap(Lp[:], 0, (16, B), (4, 4), (0, 4)),
                            _ap(Z[:], 0, (16, B), (0, 4), (1, 4)), op=A.mult)
    nc.gpsimd.tensor_tensor(u1[:], _ap(Lp[:], 1, (16, B), (4, 4), (0, 4)),
                            _ap(Z[:], 4, (16, B), (0, 4), (1, 4)), op=A.mult)
    nc.gpsimd.tensor_add(ua[:], u0[:], u1[:])
    nc.gpsimd.tensor_tensor(u0[:], _ap(Lp[:], 2, (16, B), (4, 4), (0, 4)),
                            _ap(Z[:], 8, (16, B), (0, 4), (1, 4)), op=A.mult)
    nc.gpsimd.tensor_tensor(u1[:], _ap(Lp[:], 3, (16, B), (4, 4), (0, 4)),
                            _ap(Z[:], 12, (16, B), (0, 4), (1, 4)), op=A.mult)
    nc.gpsimd.tensor_add(ub_t[:], u0[:], u1[:])
    nc.gpsimd.tensor_add(Hm[:], ua[:], ub_t[:])

    # ---- kappa / lambda ----
    Gs = T("Gs", B, 16)
    nc.vector.tensor_tensor(Gs[:], Gm[:], _ap(cst[:], 0, (0, B), (1, 16)), op=A.mult)
    Hs = T("Hs", B, 16)
    nc.gpsimd.tensor_tensor(Hs[:], Hm[:], _ap(cst[:], 16, (0, B), (1, 16)), op=A.mult)
    kl = T("kl", B, 2, 4)
    for a in range(4):
        off, sA_, sB_ = KPOS_AP[a]
        nc.vector.tensor_reduce(out=_ap(kl[:], a, (8, B), (0, 1)),
                                in_=_ap(Gs[:], off, (16, B), (sA_, 2), (sB_, 2)),
                                axis=mybir.AxisListType.XY, op=A.add)
        nc.vector.tensor_reduce(out=_ap(kl[:], 4 + a, (8, B), (0, 1)),
                                in_=_ap(Hs[:], off, (16, B), (sA_, 2), (sB_, 2)),
                                axis=mybir.AxisListType.XY, op=A.add)

    # ---- S6: assembly ----
    pr6 = T("pr6", B, 2, 3)
    nc.vector.tensor_tensor(pr6[:], w[:], _ap(kl[:], 1, (8, B), (4, 2), (1, 3)),
                            op=A.mult)
    dot = T("dot", B, 2)
    nc.vector.tensor_reduce(out=_ap(dot[:], 0, (2, B), (1, 2), (0, 1)),
                            in_=pr6[:], axis=mybir.AxisListType.X, op=A.add)
    t6a = T("t6a", B, 2)
    nc.vector.tensor_tensor(t6a[:], s1t[:], _ap(kl[:], 0, (8, B), (4, 2)), op=A.mult)
    t6b = T("t6b", B, 2)
    nc.vector.tensor_mul(t6b[:], s2t[:], dot[:])
    Aq = T("Aq", B, 2)
    nc.vector.tensor_add(Aq[:], t6a[:], t6b[:])
    tm1 = T("tm1", B, 2, 6)
    nc.vector.tensor_tensor(tm1[:], _ap(Aq[:], 0, (1, 2 * B), (0, 6)),
                            _ap(w[:], 0, (3, 2 * B), (1, 3), (0, 2)), op=A.mult)
    tm2 = T("tm2", B, 2, 6)
    nc.vector.tensor_tensor(tm2[:], _ap(snc[:], 0, (1, 2 * B), (0, 6)),
                            _ap(kl[:], 1, (4, 2 * B), (1, 3), (0, 2)), op=A.mult)
    tsum = T("tsum", B, 2, 6)
    nc.vector.tensor_add(tsum[:], tm1[:], tm2[:])
    tsgn = T("tsgn", B, 2, 6)
    nc.vector.tensor_tensor(tsgn[:], tsum[:],
                            _ap(cst[:], 32, (0, B), (6, 2), (1, 6)), op=A.mult)
    res = T("res", B, 6)
    nc.vector.tensor_add(res[:], tsgn[:, :, 0, :], tsgn[:, :, 1, :])
    nc.sync.dma_start(res_d, res[:].rearrange("p a b -> p (a b)"))


# ---------------- SPMD module build + host wrapper ----------------
_CACHE = {}


def _build_nc():
    nc = bacc.Bacc("TRN2", target_bir_lowering=False)
    ab_d = nc.dram_tensor("ab", [P, B * 2 * 3], DT, kind="ExternalInput")
    cb_d = nc.dram_tensor("cb", [P, B * 16], DT, kind="ExternalInput")
    ub_d = nc.dram_tensor("ub", [P, B * 16], DT, kind="ExternalInput")
    pp_d = nc.dram_tensor("pp", [P, B * 2], DT, kind="ExternalInput")
    cst_d = nc.dram_tensor("cst", [1, NCONST], DT, kind="ExternalInput")
    res_d = nc.dram_tensor("res", [P, B * 6], DT, kind="ExternalOutput")
    from contextlib import ExitStack
    with tile.TileContext(nc) as tc:
        with ExitStack() as ctx:
            tile_body(ctx, tc, [res_d[:]], [ab_d[:], cb_d[:], ub_d[:], pp_d[:], cst_d[:]])
    if not nc.is_finalized():
        nc.finalize()
    return nc


def _prep_in_maps(chi, cov, upd, pcpa):
    g = chi.shape[0]
    k4 = cov.shape[0] // 4
    idx = np.arange(g)
    C = cov.reshape(k4, 4, k4, 4)[idx, :, idx, :].reshape(g, 16).astype(F32)
    U = upd.reshape(k4, 4, k4, 4)[idx, :, idx, :].reshape(g, 16).astype(F32)
    alpha = np.stack([chi[:, 4], -chi[:, 2], -chi[:, 3]], axis=1).astype(F32)
    beta = np.stack([chi[:, 5], -chi[:, 1], chi[:, 0]], axis=1).astype(F32)
    pe = pcpa[0::2].astype(F32)
    po = pcpa[1::2].astype(F32)
    cst = _const_row()
    in_maps = []
    for core in range(NCORES):
        sl = slice(core * GPC, (core + 1) * GPC)
        ab = np.empty((P, B, 2, 3), F32)
        ab[:, :, 0, :] = alpha[sl].reshape(B, P, 3).transpose(1, 0, 2)
        ab[:, :, 1, :] = beta[sl].reshape(B, P, 3).transpose(1, 0, 2)
        cb = np.ascontiguousarray(C[sl].reshape(B, P, 16).transpose(1, 0, 2))
        ubm = np.ascontiguousarray(U[sl].reshape(B, P, 16).transpose(1, 0, 2))
        pp = np.ascontiguousarray(
            np.stack([pe[sl].reshape(B, P).T, po[sl].reshape(B, P).T], axis=-1))
        in_maps.append({
            "ab": ab.reshape(P, B * 2 * 3),
            "cb": cb.reshape(P, B * 16),
            "ub": ubm.reshape(P, B * 16),
            "pp": pp.reshape(P, B * 2),
            "cst": cst,
        })
    return in_maps


def _assemble(results, g):
    out = np.zeros((6, g), F32)
    for core in range(NCORES):
        res = results[core]["res"].reshape(P, B, 6)
        sl = slice(core * GPC, (core + 1) * GPC)
        for t in range(6):
            out[MPRIME[t], sl] = res[:, :, t].T.reshape(GPC)
    return out


def run_spmd(inputs, trace=False, **kw):
    """Run on the 8 neuron cores; returns (out (6,g) f32, BassKernelResults)."""
    if "nc" not in _CACHE:
        _CACHE["nc"] = _build_nc()
    nc = _CACHE["nc"]
    chi = np.asarray(inputs["chi"], F32)
    cov = np.asarray(inputs["covariance_matrix"], F32)
    upd = np.asarray(inputs["update_matrix"], F32)
    pcpa = np.asarray(inputs["partial_cost_partial_activation"], F32)
    in_maps = _prep_in_maps(chi, cov, upd, pcpa)
    br = run_bass_kernel_spmd(nc, in_maps, core_ids=list(range(NCORES)),
                              trace=trace, **kw)
    out = _assemble(br.results, chi.shape[0])
    return out, br


def kernel(**inputs) -> np.ndarray:
    out, _ = run_spmd(inputs, trace=False)
    return out
